# revision 1
# baseline (speedup 1.0000x reference)
"""Trainium2 Bass kernel for CompactPiecewiseLinearEmbeddings.

out[n, f*8+d] = sum_b h[n,f,b] * W[f,b,d] + b[f,d]
h = piecewise-linear encoding of x[n,f] over per-feature bins
    (first bin clamp_max(1), middle clamp(0,1), last bin clamp_min(0)).

Strategy (per core; data-parallel over N across 8 cores):
 - Host pre-transposes x into a padded [512, NS] layout: 16 groups of 32
   partition rows, each [16 features, ones-row, 15 pad].
 - PE float32r matmul computes the per-bin affine directly:
   s[f,j] = winv[f,j]*x[f] - e[f,j]*winv[f,j]  (ones-row carries the bias),
   packed flat 128 bin-rows per tile, 6 tiles per 16-feature group.
 - Clamp s -> h (bf16) via two static routes (no GPSIMD: its tensor ops
   run ~10x slower than DVE):
     A: DVE dual-op tensor_scalar (max(maxv), min(minv)) from PSUM.
     B: ACT Relu from PSUM (lower clamp; bin0 rows use the negated
        relu((e1-x)*winv) form with +W0 folded into the output bias),
        then a DVE 2x min-pass (SBUF bf16).
 - PE bf16 matmul contracts h (128 rows) against the block-diagonal W
   into [128 fd, n] PSUM tiles (6 accumulating matmuls per group).
 - ACT evacuates + adds output bias -> bf16 SBUF -> DMA out in [fd, n]
   layout. Host transposes/casts to the final [n, fd] f32.
"""
import numpy as np
import ml_dtypes

from concourse import bacc, mybir
from concourse.tile import TileContext
from concourse.bass_utils import run_bass_kernel_spmd

N, F, B, D = 16384, 256, 48, 8
NCORES = 8
NS = N // NCORES          # 2048 rows per core
CH = 512                  # matmul free dim / PSUM bank
NCH = NS // CH            # 4 chunks
NG = 16                   # feature groups (16 features each)
TPG = 6                   # h-tiles per group (16*48/128)
NT = NG * TPG             # 96 h-tiles
BIG = 1e30                # "no clamp" bound

# Static clamp route per h-tile index: A = DVE full clamp, B = ACT relu
# + DVE min-pass.  ~27% A / 73% B balances DVE vs ACT.
ROUTE_A_MOD = 15
ROUTE_A_CNT = 4


def tile_route_a(gi, cp=0):
    return (gi % ROUTE_A_MOD) < ROUTE_A_CNT


_cache = {}


def build_nc():
    nc = bacc.Bacc("TRN2")
    f32, bf16 = mybir.dt.float32, mybir.dt.bfloat16
    f32r = mybir.dt.float32r

    xT_ext = nc.declare_dram_parameter("xT", [4 * 128, NS], f32r, isOutput=False)
    selA_ext = nc.declare_dram_parameter("selA", [128, 4 * TPG * 128], f32r, isOutput=False)
    selB_ext = nc.declare_dram_parameter("selB", [128, 4 * TPG * 128], f32r, isOutput=False)
    wpk_ext = nc.declare_dram_parameter("wpack", [128, NT * 128], bf16, isOutput=False)
    obias_ext = nc.declare_dram_parameter("obias", [128, NG], f32, isOutput=False)
    maxv_ext = nc.declare_dram_parameter("maxv", [128, NT], f32, isOutput=False)
    minv_ext = nc.declare_dram_parameter("minv", [128, NT], f32, isOutput=False)
    out_ext = nc.declare_dram_parameter("out", [F * D, NS], bf16, isOutput=True)

    Ident = mybir.ActivationFunctionType.Identity
    Relu = mybir.ActivationFunctionType.Relu
    amax, amin = mybir.AluOpType.max, mybir.AluOpType.min

    with TileContext(nc) as tc:
        with (
            tc.tile_pool(name="const", bufs=1) as cpool,
            tc.tile_pool(name="hbuf", bufs=6) as hpool,
            tc.tile_pool(name="sbuf2", bufs=4) as spool,
            tc.tile_pool(name="osb", bufs=4) as opool,
            tc.tile_pool(name="bc", bufs=2, space="PSUM") as bcpool,
            tc.tile_pool(name="oc", bufs=2, space="PSUM") as ocpool,
        ):
            # ---- constants ----
            xT = [cpool.tile([128, NS], f32r, tag=f"xT{i}", name=f"xT{i}")
                  for i in range(4)]
            for i in range(4):
                nc.sync.dma_start(out=xT[i][:], in_=xT_ext[i * 128:(i + 1) * 128, :])
            selA = cpool.tile([128, 4 * TPG * 128], f32r)
            selB = cpool.tile([128, 4 * TPG * 128], f32r)
            wpk = cpool.tile([128, NT * 128], bf16)
            obias = cpool.tile([128, NG], f32)
            maxv = cpool.tile([128, NT], f32)
            minv = cpool.tile([128, NT], f32)
            for t, e in [(selA, selA_ext), (selB, selB_ext), (wpk, wpk_ext),
                         (obias, obias_ext), (maxv, maxv_ext),
                         (minv, minv_ext)]:
                nc.sync.dma_start(out=t[:], in_=e[:])

            # ---- main loop: chunk-pairs (1024 cols) ----
            for cp in range(NCH // 2):
                for g in range(NG):
                    q = g % 4
                    xt = xT[g // 4]
                    oc = ocpool.tile([128, 2 * CH], f32, tag="oc")
                    for t in range(TPG):
                        gi = g * TPG + t
                        ra = tile_route_a(gi, cp)
                        sel = selA if ra else selB
                        selcol = ((g // 4) * TPG + t) * 128
                        ps = bcpool.tile([128, 2 * CH], f32, tag="ps")
                        for half in range(2):
                            c = 2 * cp + half
                            nc.tensor.matmul(
                                ps[:, half * CH:(half + 1) * CH],
                                sel[32 * q:32 * q + 32, selcol:selcol + 128],
                                xt[32 * q:32 * q + 32, c * CH:(c + 1) * CH],
                                start=True, stop=True,
                                tile_position=(32 * q, 0),
                            )
                        h = hpool.tile([128, 2 * CH], bf16, tag="h")
                        if ra:
                            nc.vector.tensor_scalar(
                                h[:], ps[:], maxv[:, gi:gi + 1], minv[:, gi:gi + 1],
                                amax, amin,
                            )
                        else:
                            r = spool.tile([128, 2 * CH], bf16, tag="r")
                            nc.scalar.activation(r[:], ps[:], Relu)
                            nc.vector.tensor_scalar(
                                h[:], r[:], minv[:, gi:gi + 1], None, amin,
                            )
                        for half in range(2):
                            nc.tensor.matmul(
                                oc[:, half * CH:(half + 1) * CH],
                                wpk[:, gi * 128:(gi + 1) * 128],
                                h[:, half * CH:(half + 1) * CH],
                                start=(t == 0), stop=(t == TPG - 1),
                            )
                    osb = opool.tile([128, 2 * CH], bf16, tag="osb")
                    nc.scalar.activation(osb[:], oc[:], Ident,
                                         bias=obias[:, g:g + 1])
                    nc.sync.dma_start(
                        out=out_ext[g * 128:(g + 1) * 128,
                                    2 * cp * CH:2 * (cp + 1) * CH],
                        in_=osb[:])

    nc.compile()
    return nc


def host_constants(edges, width, W, b):
    """Build packed constant tensors. edges/width [F,B], W [F,B,D], b [F,D]."""
    f32 = np.float32
    edges = np.asarray(edges, f32)
    width = np.asarray(width, f32)
    W = np.asarray(W, f32).copy()
    b = np.asarray(b, f32)
    winv = (1.0 / width).astype(f32)
    e1 = edges[:, 0] + width[:, 0]   # second boundary

    # selA/selB: [128 partitions, 4*TPG*128]; band q=rows 32q..32q+31
    # serves groups with g%4==q: [16 feats, ones@16, 15 pad].
    selA = np.zeros((128, 4 * TPG * 128), f32)
    selB = np.zeros((128, 4 * TPG * 128), f32)
    wpack = np.zeros((128, NT * 128), f32)
    obias = np.zeros((128, NG), f32)
    maxv = np.zeros((128, NT), f32)
    minv = np.zeros((128, NT), f32)

    # which (f,) bin0 rows sit in route-B tiles for any chunk-pair?
    # route depends on cp; bin0's W-sign flip must be cp-independent ->
    # force bin0/bin47 handling identical across routes:
    #   route A tile: bin0 row uses s-form (maxv=-BIG, minv=1), weight +W0.
    #   route B tile: bin0 row uses relu-form (value relu(e1-x)*winv),
    #     weight -W0, obias += W0.
    # A tile's route must therefore be the same for every cp: make
    # tile_route_a depend only on gi.  (checked below)
    for cp in range(NCH // 2):
        for gi in range(NT):
            assert tile_route_a(gi, cp) == tile_route_a(gi, 0), \
                "route must be cp-independent (bin0 weight sign)"

    for g in range(NG):
        q, blk = g % 4, g // 4
        for t in range(TPG):
            gi = g * TPG + t
            ra = tile_route_a(gi, 0)
            selcol = (blk * TPG + t) * 128
            for m in range(128):
                r = 128 * t + m          # row within the group (0..767)
                fl, j = r // B, r % B    # local feature, bin
                f = 16 * g + fl
                wv, ev = winv[f, j], edges[f, j]
                # default (middle bins), s-form: s = (x-e)*winv
                selA[32 * q + fl, selcol + m] = wv
                selA[32 * q + 16, selcol + m] = -ev * wv
                selB[32 * q + fl, selcol + m] = wv
                selB[32 * q + 16, selcol + m] = -ev * wv
                wcoef = W[f, j, :]
                if j == 0:
                    if ra:
                        maxv[m, gi] = -BIG
                        minv[m, gi] = 1.0
                    else:
                        # relu-form: value = relu((e1-x)*winv), weight -W0,
                        # obias += W0
                        selB[32 * q + fl, selcol + m] = -wv
                        selB[32 * q + 16, selcol + m] = e1[f] * wv
                        minv[m, gi] = BIG
                        wcoef = -W[f, j, :]
                        obias[8 * fl:8 * fl + 8, g] += W[f, j, :]
                elif j == B - 1:
                    maxv[m, gi] = 0.0
                    minv[m, gi] = BIG
                else:
                    maxv[m, gi] = 0.0
                    minv[m, gi] = 1.0
                wpack[m, gi * 128 + 8 * fl:gi * 128 + 8 * fl + 8] = wcoef
        for fl in range(16):
            obias[8 * fl:8 * fl + 8, g] += b[16 * g + fl, :]

    return {
        "selA": selA,
        "selB": selB,
        "wpack": wpack.astype(ml_dtypes.bfloat16),
        "obias": obias,
        "maxv": maxv,
        "minv": minv,
    }


def make_xT(x_core):
    """x_core [NS, F] f32 -> padded transposed [512, NS] f32."""
    xT = np.zeros((4 * 128, NS), np.float32)
    xt_full = np.ascontiguousarray(x_core.T)          # [F, NS]
    for g in range(NG):
        base = 32 * (g % 4) + 128 * (g // 4)
        xT[base:base + 16, :] = xt_full[16 * g:16 * g + 16, :]
        xT[base + 16, :] = 1.0
    return xT


def make_in_maps(x, edges, width, W, b):
    consts = host_constants(edges, width, W, b)
    x = np.ascontiguousarray(np.asarray(x, dtype=np.float32))
    in_maps = []
    for core in range(NCORES):
        m = dict(consts)
        m["xT"] = make_xT(x[core * NS:(core + 1) * NS, :])
        in_maps.append(m)
    return in_maps


def kernel(x, edges, width, W, b):
    if "nc" not in _cache:
        _cache["nc"] = build_nc()
    nc = _cache["nc"]
    in_maps = make_in_maps(x, edges, width, W, b)
    res = run_bass_kernel_spmd(nc, in_maps, core_ids=list(range(NCORES)))
    outs = []
    for r in res.results:
        o = np.asarray(r["out"])                      # [F*D, NS] bf16
        outs.append(o.astype(np.float32).T)           # [NS, F*D]
    return np.ascontiguousarray(np.concatenate(outs, axis=0))



# revision 2
# speedup vs baseline: 1.0942x; 1.0942x over previous
"""Trainium2 Bass kernel for CompactPiecewiseLinearEmbeddings.

out[n, f*8+d] = sum_b h[n,f,b] * W[f,b,d] + b[f,d]
h = piecewise-linear encoding of x[n,f] over per-feature bins
    (first bin clamp_max(1), middle clamp(0,1), last bin clamp_min(0)).

Strategy (per core; data-parallel over N across 8 cores):
 - All-bf16 PE path (f32r moving data caps the PE clock at 1.2 GHz;
   pure-bf16 streams sustain 2.4 GHz).  x is split hi/lo into two bf16
   rows per feature (bf16*bf16 products are exact in the fp32 PSUM
   accumulate), and the per-bin bias -e*winv is split across two bf16
   ones-rows, so stage-1 matches f32r accuracy.
 - Host packs x into 8 xT tiles [128, NS]: two 34-row bands per tile
   (rows 0/64 +: 16 x_hi, 16 x_lo, ones, ones) serving groups (2i,2i+1).
 - Stage-1 bf16 matmul per (group, tile): s[f,j] = winv*x - e*winv,
   128 bin-rows x 1024 cols, band contraction 34 rows via tile_position.
 - Clamp s -> h (bf16) via two static routes:
     A (~37%): DVE dual tensor_scalar (max, min) from PSUM.
     B: ACT Relu from PSUM (bin0 rows use the negated relu((e1-x)*winv)
        form with +W0 folded into the output bias), then DVE min-pass.
 - Stage-2 bf16 matmul contracts h against block-diagonal W into
   [128 fd, 1024] PSUM (6 accumulating matmuls per group).
 - Output bias+evac split ACT/DVE (3:1) -> bf16 SBUF -> DMA out in
   [fd, n] layout.  Host transposes/casts to the final [n, fd] f32.
"""
import numpy as np
import ml_dtypes

from concourse import bacc, mybir
from concourse.tile import TileContext
from concourse.bass_utils import run_bass_kernel_spmd

N, F, B, D = 16384, 256, 48, 8
NCORES = 8
NS = N // NCORES          # 2048 rows per core
CH = 512                  # matmul free dim (PSUM bank = 512 f32)
NG = 16                   # feature groups (16 features each)
TPG = 6                   # h-tiles per group (16*48/128)
NT = NG * TPG             # 96 h-tiles
NB = 34                   # band rows: 16 x_hi + 16 x_lo + 2 ones
BIG = 1e30

BF16 = ml_dtypes.bfloat16


def tile_route_a(gi):
    """Static clamp route per h-tile: A = DVE dual clamp, B = ACT relu
    + DVE min.  ~37% A balances DVE vs ACT."""
    return (gi % 11) < 4


def bfr(a):
    """Round f32 array to bf16 grid, keep f32."""
    return np.asarray(a, np.float32).astype(BF16).astype(np.float32)


_cache = {}


def build_nc():
    nc = bacc.Bacc("TRN2")
    f32, bf16 = mybir.dt.float32, mybir.dt.bfloat16

    xT_ext = nc.declare_dram_parameter("xT", [8 * 128, NS], bf16, isOutput=False)
    selpk_ext = nc.declare_dram_parameter("selpk", [128, 8 * TPG * 128], bf16,
                                          isOutput=False)
    wpk_ext = nc.declare_dram_parameter("wpack", [128, NT * 128], bf16,
                                        isOutput=False)
    obias_ext = nc.declare_dram_parameter("obias", [128, NG], f32, isOutput=False)
    maxv_ext = nc.declare_dram_parameter("maxv", [128, NT], f32, isOutput=False)
    minv_ext = nc.declare_dram_parameter("minv", [128, NT], f32, isOutput=False)
    out_ext = nc.declare_dram_parameter("out", [F * D, NS], bf16, isOutput=True)

    Ident = mybir.ActivationFunctionType.Identity
    Relu = mybir.ActivationFunctionType.Relu
    amax, amin = mybir.AluOpType.max, mybir.AluOpType.min
    aadd = mybir.AluOpType.add

    with TileContext(nc) as tc:
        with (
            tc.tile_pool(name="const", bufs=1) as cpool,
            tc.tile_pool(name="hbuf", bufs=6) as hpool,
            tc.tile_pool(name="sbuf2", bufs=4) as spool,
            tc.tile_pool(name="osb", bufs=4) as opool,
            tc.tile_pool(name="bc", bufs=2, space="PSUM") as bcpool,
            tc.tile_pool(name="oc", bufs=2, space="PSUM") as ocpool,
        ):
            # ---- constants ----
            xT = [cpool.tile([128, NS], bf16, tag=f"xT{i}", name=f"xT{i}")
                  for i in range(8)]
            for i in range(8):
                nc.sync.dma_start(out=xT[i][:], in_=xT_ext[i * 128:(i + 1) * 128, :])
            selpk = cpool.tile([128, 8 * TPG * 128], bf16)
            wpk = cpool.tile([128, NT * 128], bf16)
            obias = cpool.tile([128, NG], f32)
            maxv = cpool.tile([128, NT], f32)
            minv = cpool.tile([128, NT], f32)
            for t, e in [(selpk, selpk_ext), (wpk, wpk_ext),
                         (obias, obias_ext), (maxv, maxv_ext),
                         (minv, minv_ext)]:
                nc.sync.dma_start(out=t[:], in_=e[:])

            # ---- main loop: 1024-col chunks ----
            for cp in range(NS // (2 * CH)):
                for g in range(NG):
                    ti, band = g // 2, 64 * (g % 2)
                    xt = xT[ti]
                    oc = ocpool.tile([128, 2 * CH], f32, tag="oc")
                    for t in range(TPG):
                        gi = g * TPG + t
                        ra = tile_route_a(gi)
                        selcol = (ti * TPG + t) * 128
                        ps = bcpool.tile([128, 2 * CH], f32, tag="ps")
                        for half in range(2):
                            c = 2 * cp + half
                            nc.tensor.matmul(
                                ps[:, half * CH:(half + 1) * CH],
                                selpk[band:band + NB, selcol:selcol + 128],
                                xt[band:band + NB, c * CH:(c + 1) * CH],
                                start=True, stop=True,
                                tile_position=(band, 0),
                            )
                        h = hpool.tile([128, 2 * CH], bf16, tag="h")
                        if ra:
                            nc.vector.tensor_scalar(
                                h[:], ps[:], maxv[:, gi:gi + 1], minv[:, gi:gi + 1],
                                amax, amin,
                            )
                        else:
                            r = spool.tile([128, 2 * CH], bf16, tag="r")
                            nc.scalar.activation(r[:], ps[:], Relu)
                            nc.vector.tensor_scalar(
                                h[:], r[:], minv[:, gi:gi + 1], None, amin,
                            )
                        for half in range(2):
                            nc.tensor.matmul(
                                oc[:, half * CH:(half + 1) * CH],
                                wpk[:, gi * 128:(gi + 1) * 128],
                                h[:, half * CH:(half + 1) * CH],
                                start=(t == 0), stop=(t == TPG - 1),
                            )
                    osb = opool.tile([128, 2 * CH], bf16, tag="osb")
                    if g % 4 == 3:
                        nc.vector.tensor_scalar(osb[:], oc[:],
                                                obias[:, g:g + 1], None, aadd)
                    else:
                        nc.scalar.activation(osb[:], oc[:], Ident,
                                             bias=obias[:, g:g + 1])
                    nc.sync.dma_start(
                        out=out_ext[g * 128:(g + 1) * 128,
                                    2 * cp * CH:2 * (cp + 1) * CH],
                        in_=osb[:])

    nc.compile()
    return nc


def host_constants(edges, width, W, b):
    """Build packed constant tensors. edges/width [F,B], W [F,B,D], b [F,D]."""
    f32 = np.float32
    edges = np.asarray(edges, f32)
    width = np.asarray(width, f32)
    W = np.asarray(W, f32).copy()
    b = np.asarray(b, f32)
    wv_all = bfr(1.0 / width)        # bf16-valued winv, f32
    e1 = edges[:, 0] + width[:, 0]   # second boundary

    selpk = np.zeros((128, 8 * TPG * 128), f32)
    wpack = np.zeros((128, NT * 128), f32)
    obias = np.zeros((128, NG), f32)
    maxv = np.zeros((128, NT), f32)
    minv = np.zeros((128, NT), f32)

    for g in range(NG):
        ti, band = g // 2, 64 * (g % 2)
        for t in range(TPG):
            gi = g * TPG + t
            ra = tile_route_a(gi)
            selcol = (ti * TPG + t) * 128
            for m in range(128):
                r = 128 * t + m          # row within the group (0..767)
                fl, j = r // B, r % B    # local feature, bin
                f = 16 * g + fl
                wv = wv_all[f, j]
                wcoef = W[f, j, :]
                if j == 0 and not ra:
                    # relu-form: value = relu((e1-x)*winv), weight -W0,
                    # obias += W0
                    xw = -wv
                    cval = f32(e1[f] * wv)
                    minv[m, gi] = BIG
                    wcoef = -W[f, j, :]
                    obias[8 * fl:8 * fl + 8, g] += W[f, j, :]
                else:
                    xw = wv
                    cval = f32(-edges[f, j] * wv)
                    if j == 0:           # route A bin0: min(s,1) only
                        maxv[m, gi] = -BIG
                        minv[m, gi] = 1.0
                    elif j == B - 1:     # last bin: max(s,0) only
                        maxv[m, gi] = 0.0
                        minv[m, gi] = BIG
                    else:
                        maxv[m, gi] = 0.0
                        minv[m, gi] = 1.0
                chi = bfr(cval)
                clo = f32(cval - chi)
                selpk[band + fl, selcol + m] = xw
                selpk[band + 16 + fl, selcol + m] = xw
                selpk[band + 32, selcol + m] = chi
                selpk[band + 33, selcol + m] = clo
                wpack[m, gi * 128 + 8 * fl:gi * 128 + 8 * fl + 8] = wcoef
        for fl in range(16):
            obias[8 * fl:8 * fl + 8, g] += b[16 * g + fl, :]

    return {
        "selpk": selpk.astype(BF16),
        "wpack": wpack.astype(BF16),
        "obias": obias,
        "maxv": maxv,
        "minv": minv,
    }


def make_xT(x_core):
    """x_core [NS, F] f32 -> packed [8*128, NS] bf16 (hi/lo split bands)."""
    xT = np.zeros((8 * 128, NS), BF16)
    xt_full = np.ascontiguousarray(x_core.T).astype(np.float32)   # [F, NS]
    xhi = xt_full.astype(BF16)
    xlo = (xt_full - xhi.astype(np.float32)).astype(BF16)
    one = BF16(1.0)
    for g in range(NG):
        base = 128 * (g // 2) + 64 * (g % 2)
        xT[base:base + 16, :] = xhi[16 * g:16 * g + 16, :]
        xT[base + 16:base + 32, :] = xlo[16 * g:16 * g + 16, :]
        xT[base + 32, :] = one
        xT[base + 33, :] = one
    return xT


def make_in_maps(x, edges, width, W, b):
    consts = host_constants(edges, width, W, b)
    x = np.ascontiguousarray(np.asarray(x, dtype=np.float32))
    in_maps = []
    for core in range(NCORES):
        m = dict(consts)
        m["xT"] = make_xT(x[core * NS:(core + 1) * NS, :])
        in_maps.append(m)
    return in_maps


def kernel(x, edges, width, W, b):
    if "nc" not in _cache:
        _cache["nc"] = build_nc()
    nc = _cache["nc"]
    in_maps = make_in_maps(x, edges, width, W, b)
    res = run_bass_kernel_spmd(nc, in_maps, core_ids=list(range(NCORES)))
    outs = []
    for r in res.results:
        o = np.asarray(r["out"])                      # [F*D, NS] bf16
        outs.append(o.astype(np.float32).T)           # [NS, F*D]
    return np.ascontiguousarray(np.concatenate(outs, axis=0))


# revision 4
# speedup vs baseline: 1.5466x; 1.4134x over previous
"""Trainium2 Bass kernel for CompactPiecewiseLinearEmbeddings.

out[n, f*8+d] = sum_b h[n,f,b] * W[f,b,d] + b[f,d]
h = piecewise-linear encoding of x[n,f] over per-feature bins
    (first bin clamp_max(1), middle clamp(0,1), last bin clamp_min(0)).

Strategy (per core; data-parallel over N across 8 cores):
 - All-bf16 PE path (f32r moving data caps the PE clock at 1.2 GHz;
   pure-bf16 streams sustain 2.4 GHz).  x is split hi/lo into two bf16
   rows per feature (bf16*bf16 products are exact in the fp32 PSUM
   accumulate), and the per-bin bias -e*winv is split across two bf16
   ones-rows, so stage-1 matches f32r accuracy.
 - Host packs x into 8 xT tiles [128, NS]: two 34-row bands per tile
   (rows 0/64 +: 16 x_hi, 16 x_lo, ones, ones) serving groups (2i,2i+1).
 - Stage-1 bf16 matmul per (group, tile): s[f,j] = winv*x - e*winv.
   Contraction is always the full 128 partitions with zeros in the
   unused weight rows: mixing partial-band (tile_position) matmuls with
   full-128 ones drops the PE cadence from 216ns to ~322ns per matmul
   (measured), while uniform [128,128,512] shapes sustain the 2.4 GHz
   boost.
 - Clamp s -> h (bf16) via two static routes:
     A (~37%): DVE dual tensor_scalar (max, min) from PSUM.
     B: ACT Relu from PSUM (bin0 rows use the negated relu((e1-x)*winv)
        form with +W0 folded into the output bias), then DVE min-pass.
 - Stage-2 bf16 matmul contracts h against block-diagonal W into
   [128 fd, 1024] PSUM (6 accumulating matmuls per group).
 - Output bias+evac split ACT/DVE (3:1) -> bf16 SBUF -> DMA out in
   [fd, n] layout.  Host transposes/casts to the final [n, fd] f32.
"""
import numpy as np
import ml_dtypes

from concourse import bacc, mybir
from concourse.tile import TileContext
from concourse.bass_utils import run_bass_kernel_spmd

N, F, B, D = 16384, 256, 48, 8
NCORES = 8
NS = N // NCORES          # 2048 rows per core
CH = 512                  # matmul free dim (PSUM bank = 512 f32)
NG = 16                   # feature groups (16 features each)
TPG = 6                   # h-tiles per group (16*48/128)
NT = NG * TPG             # 96 h-tiles
NB = 34                   # band rows: 16 x_hi + 16 x_lo + 2 ones
BIG = 1e30

BF16 = ml_dtypes.bfloat16


def tile_route_a(gi):
    """Static clamp route per h-tile: A = DVE dual clamp, B = ACT relu
    + DVE min.  ~37% A balances DVE vs ACT."""
    return (gi % 11) < 4


def bfr(a):
    """Round f32 array to bf16 grid, keep f32."""
    return np.asarray(a, np.float32).astype(BF16).astype(np.float32)


_cache = {}


def build_nc():
    nc = bacc.Bacc("TRN2")
    f32, bf16 = mybir.dt.float32, mybir.dt.bfloat16

    xT_ext = nc.declare_dram_parameter("xT", [8 * 128, NS], bf16, isOutput=False)
    selpk_ext = nc.declare_dram_parameter("selpk", [128, NT * 128], bf16,
                                          isOutput=False)
    wpk_ext = nc.declare_dram_parameter("wpack", [128, NT * 128], bf16,
                                        isOutput=False)
    obias_ext = nc.declare_dram_parameter("obias", [128, NG], f32, isOutput=False)
    maxv_ext = nc.declare_dram_parameter("maxv", [128, NT], f32, isOutput=False)
    minv_ext = nc.declare_dram_parameter("minv", [128, NT], f32, isOutput=False)
    out_ext = nc.declare_dram_parameter("out", [F * D, NS], bf16, isOutput=True)

    Ident = mybir.ActivationFunctionType.Identity
    Relu = mybir.ActivationFunctionType.Relu
    amax, amin = mybir.AluOpType.max, mybir.AluOpType.min
    aadd = mybir.AluOpType.add

    with TileContext(nc) as tc:
        with (
            tc.tile_pool(name="const", bufs=1) as cpool,
            tc.tile_pool(name="hbuf", bufs=6) as hpool,
            tc.tile_pool(name="sbuf2", bufs=4) as spool,
            tc.tile_pool(name="osb", bufs=4) as opool,
            tc.tile_pool(name="bc", bufs=2, space="PSUM") as bcpool,
            tc.tile_pool(name="oc", bufs=2, space="PSUM") as ocpool,
        ):
            # ---- constants ----
            xT = [cpool.tile([128, NS], bf16, tag=f"xT{i}", name=f"xT{i}")
                  for i in range(8)]
            for i in range(8):
                nc.sync.dma_start(out=xT[i][:], in_=xT_ext[i * 128:(i + 1) * 128, :])
            selpk = cpool.tile([128, NT * 128], bf16)
            wpk = cpool.tile([128, NT * 128], bf16)
            obias = cpool.tile([128, NG], f32)
            maxv = cpool.tile([128, NT], f32)
            minv = cpool.tile([128, NT], f32)
            for t, e in [(selpk, selpk_ext), (wpk, wpk_ext),
                         (obias, obias_ext), (maxv, maxv_ext),
                         (minv, minv_ext)]:
                nc.sync.dma_start(out=t[:], in_=e[:])

            # ---- main loop: 1024-col chunks ----
            for cp in range(NS // (2 * CH)):
                for g in range(NG):
                    ti, band = g // 2, 64 * (g % 2)
                    xt = xT[ti]
                    oc = ocpool.tile([128, 2 * CH], f32, tag="oc")
                    for t in range(TPG):
                        gi = g * TPG + t
                        ra = tile_route_a(gi)
                        ps = bcpool.tile([128, 2 * CH], f32, tag="ps")
                        for half in range(2):
                            c = 2 * cp + half
                            nc.tensor.matmul(
                                ps[:, half * CH:(half + 1) * CH],
                                selpk[:, gi * 128:(gi + 1) * 128],
                                xt[:, c * CH:(c + 1) * CH],
                                start=True, stop=True,
                            )
                        h = hpool.tile([128, 2 * CH], bf16, tag="h")
                        if ra:
                            nc.vector.tensor_scalar(
                                h[:], ps[:], maxv[:, gi:gi + 1], minv[:, gi:gi + 1],
                                amax, amin,
                            )
                        else:
                            r = spool.tile([128, 2 * CH], bf16, tag="r")
                            nc.scalar.activation(r[:], ps[:], Relu)
                            nc.vector.tensor_scalar(
                                h[:], r[:], minv[:, gi:gi + 1], None, amin,
                            )
                        for half in range(2):
                            nc.tensor.matmul(
                                oc[:, half * CH:(half + 1) * CH],
                                wpk[:, gi * 128:(gi + 1) * 128],
                                h[:, half * CH:(half + 1) * CH],
                                start=(t == 0), stop=(t == TPG - 1),
                            )
                    osb = opool.tile([128, 2 * CH], bf16, tag="osb")
                    if g % 4 == 3:
                        nc.vector.tensor_scalar(osb[:], oc[:],
                                                obias[:, g:g + 1], None, aadd)
                    else:
                        nc.scalar.activation(osb[:], oc[:], Ident,
                                             bias=obias[:, g:g + 1])
                    nc.sync.dma_start(
                        out=out_ext[g * 128:(g + 1) * 128,
                                    2 * cp * CH:2 * (cp + 1) * CH],
                        in_=osb[:])

    nc.compile()
    return nc


def host_constants(edges, width, W, b):
    """Build packed constant tensors. edges/width [F,B], W [F,B,D], b [F,D]."""
    f32 = np.float32
    edges = np.asarray(edges, f32)
    width = np.asarray(width, f32)
    W = np.asarray(W, f32).copy()
    b = np.asarray(b, f32)
    wv_all = bfr(1.0 / width)        # bf16-valued winv, f32
    e1 = edges[:, 0] + width[:, 0]   # second boundary

    selpk = np.zeros((128, NT * 128), f32)
    wpack = np.zeros((128, NT * 128), f32)
    obias = np.zeros((128, NG), f32)
    maxv = np.zeros((128, NT), f32)
    minv = np.zeros((128, NT), f32)

    for g in range(NG):
        ti, band = g // 2, 64 * (g % 2)
        for t in range(TPG):
            gi = g * TPG + t
            ra = tile_route_a(gi)
            selcol = gi * 128
            for m in range(128):
                r = 128 * t + m          # row within the group (0..767)
                fl, j = r // B, r % B    # local feature, bin
                f = 16 * g + fl
                wv = wv_all[f, j]
                wcoef = W[f, j, :]
                if j == 0 and not ra:
                    # relu-form: value = relu((e1-x)*winv), weight -W0,
                    # obias += W0
                    xw = -wv
                    cval = f32(e1[f] * wv)
                    minv[m, gi] = BIG
                    wcoef = -W[f, j, :]
                    obias[8 * fl:8 * fl + 8, g] += W[f, j, :]
                else:
                    xw = wv
                    cval = f32(-edges[f, j] * wv)
                    if j == 0:           # route A bin0: min(s,1) only
                        maxv[m, gi] = -BIG
                        minv[m, gi] = 1.0
                    elif j == B - 1:     # last bin: max(s,0) only
                        maxv[m, gi] = 0.0
                        minv[m, gi] = BIG
                    else:
                        maxv[m, gi] = 0.0
                        minv[m, gi] = 1.0
                chi = bfr(cval)
                clo = f32(cval - chi)
                selpk[band + fl, selcol + m] = xw
                selpk[band + 16 + fl, selcol + m] = xw
                selpk[band + 32, selcol + m] = chi
                selpk[band + 33, selcol + m] = clo
                wpack[m, gi * 128 + 8 * fl:gi * 128 + 8 * fl + 8] = wcoef
        for fl in range(16):
            obias[8 * fl:8 * fl + 8, g] += b[16 * g + fl, :]

    return {
        "selpk": selpk.astype(BF16),
        "wpack": wpack.astype(BF16),
        "obias": obias,
        "maxv": maxv,
        "minv": minv,
    }


def make_xT(x_core):
    """x_core [NS, F] f32 -> packed [8*128, NS] bf16 (hi/lo split bands)."""
    xT = np.zeros((8 * 128, NS), BF16)
    xt_full = np.ascontiguousarray(x_core.T).astype(np.float32)   # [F, NS]
    xhi = xt_full.astype(BF16)
    xlo = (xt_full - xhi.astype(np.float32)).astype(BF16)
    one = BF16(1.0)
    for g in range(NG):
        base = 128 * (g // 2) + 64 * (g % 2)
        xT[base:base + 16, :] = xhi[16 * g:16 * g + 16, :]
        xT[base + 16:base + 32, :] = xlo[16 * g:16 * g + 16, :]
        xT[base + 32, :] = one
        xT[base + 33, :] = one
    return xT


def make_in_maps(x, edges, width, W, b):
    consts = host_constants(edges, width, W, b)
    x = np.ascontiguousarray(np.asarray(x, dtype=np.float32))
    in_maps = []
    for core in range(NCORES):
        m = dict(consts)
        m["xT"] = make_xT(x[core * NS:(core + 1) * NS, :])
        in_maps.append(m)
    return in_maps


def kernel(x, edges, width, W, b):
    if "nc" not in _cache:
        _cache["nc"] = build_nc()
    nc = _cache["nc"]
    in_maps = make_in_maps(x, edges, width, W, b)
    res = run_bass_kernel_spmd(nc, in_maps, core_ids=list(range(NCORES)))
    outs = []
    for r in res.results:
        o = np.asarray(r["out"])                      # [F*D, NS] bf16
        outs.append(o.astype(np.float32).T)           # [NS, F*D]
    return np.ascontiguousarray(np.concatenate(outs, axis=0))


# revision 5
# speedup vs baseline: 1.5900x; 1.0280x over previous
"""Trainium2 Bass kernel for CompactPiecewiseLinearEmbeddings.

out[n, f*8+d] = sum_b h[n,f,b] * W[f,b,d] + b[f,d]
h = piecewise-linear encoding of x[n,f] over per-feature bins
    (first bin clamp_max(1), middle clamp(0,1), last bin clamp_min(0)).

Strategy (per core; data-parallel over N across 8 cores):
 - All-bf16 PE path (f32r moving data caps the PE clock at 1.2 GHz;
   pure-bf16 streams sustain 2.4 GHz).  x is split hi/lo into two bf16
   rows per feature (bf16*bf16 products are exact in the fp32 PSUM
   accumulate), and the per-bin bias -e*winv is split across two bf16
   ones-rows, so stage-1 matches f32r accuracy.
 - Host packs x into 8 xT tiles [128, NS]: two 34-row bands per tile
   (rows 0/64 +: 16 x_hi, 16 x_lo, ones, ones) serving groups (2i,2i+1).
 - Stage-1 bf16 matmul per (group, tile): s[f,j] = winv*x - e*winv.
   Contraction is always the full 128 partitions with zeros in the
   unused weight rows: mixing partial-band (tile_position) matmuls with
   full-128 ones drops the PE cadence from 216ns to ~322ns per matmul
   (measured), while uniform [128,128,512] shapes sustain the 2.4 GHz
   boost.
 - Clamp s -> h (bf16) via two static routes:
     A (~37%): DVE dual tensor_scalar (max, min) from PSUM.
     B: ACT Relu from PSUM (bin0 rows use the negated relu((e1-x)*winv)
        form with +W0 folded into the output bias), then DVE min-pass.
 - Stage-2 bf16 matmul contracts h against block-diagonal W into
   [128 fd, 1024] PSUM (6 accumulating matmuls per group).
 - Output bias+evac split ACT/DVE (3:1) -> bf16 SBUF -> DMA out in
   [fd, n] layout.  Host transposes/casts to the final [n, fd] f32.
"""
import numpy as np
import ml_dtypes

from concourse import bacc, mybir
from concourse.tile import TileContext
from concourse.bass_utils import run_bass_kernel_spmd

N, F, B, D = 16384, 256, 48, 8
NCORES = 8
NS = N // NCORES          # 2048 rows per core
CH = 512                  # matmul free dim (PSUM bank = 512 f32)
NG = 16                   # feature groups (16 features each)
TPG = 6                   # h-tiles per group (16*48/128)
NT = NG * TPG             # 96 h-tiles
NB = 34                   # band rows: 16 x_hi + 16 x_lo + 2 ones
BIG = 1e30

BF16 = ml_dtypes.bfloat16


def tile_route_a(gi):
    """Static clamp route per h-tile: A = DVE dual clamp, B = ACT relu
    + DVE min.  ~37% A balances DVE vs ACT."""
    return (gi % 11) < 4


def bfr(a):
    """Round f32 array to bf16 grid, keep f32."""
    return np.asarray(a, np.float32).astype(BF16).astype(np.float32)


_cache = {}


def build_nc():
    nc = bacc.Bacc("TRN2")
    f32, bf16 = mybir.dt.float32, mybir.dt.bfloat16

    xT_ext = nc.declare_dram_parameter("xT", [8 * 128, NS], bf16, isOutput=False)
    selpk_ext = nc.declare_dram_parameter("selpk", [128, NT * 128], bf16,
                                          isOutput=False)
    wpk_ext = nc.declare_dram_parameter("wpack", [128, NT * 128], bf16,
                                        isOutput=False)
    obias_ext = nc.declare_dram_parameter("obias", [128, NG], f32, isOutput=False)
    maxv_ext = nc.declare_dram_parameter("maxv", [128, NT], f32, isOutput=False)
    minv_ext = nc.declare_dram_parameter("minv", [128, NT], f32, isOutput=False)
    out_ext = nc.declare_dram_parameter("out", [F * D, NS], bf16, isOutput=True)

    Ident = mybir.ActivationFunctionType.Identity
    Relu = mybir.ActivationFunctionType.Relu
    amax, amin = mybir.AluOpType.max, mybir.AluOpType.min
    aadd = mybir.AluOpType.add

    with TileContext(nc) as tc:
        with (
            tc.tile_pool(name="const", bufs=1) as cpool,
            tc.tile_pool(name="hbuf", bufs=6) as hpool,
            tc.tile_pool(name="sbuf2", bufs=4) as spool,
            tc.tile_pool(name="osb", bufs=4) as opool,
            tc.tile_pool(name="bc", bufs=2, space="PSUM") as bcpool,
            tc.tile_pool(name="oc", bufs=2, space="PSUM") as ocpool,
        ):
            # ---- constants ----
            xT = [cpool.tile([128, NS], bf16, tag=f"xT{i}", name=f"xT{i}")
                  for i in range(8)]
            for i in range(8):
                nc.sync.dma_start(out=xT[i][:], in_=xT_ext[i * 128:(i + 1) * 128, :])
            selpk = cpool.tile([128, NT * 128], bf16)
            wpk = cpool.tile([128, NT * 128], bf16)
            obias = cpool.tile([128, NG], f32)
            maxv = cpool.tile([128, NT], f32)
            minv = cpool.tile([128, NT], f32)
            for t, e in [(selpk, selpk_ext), (wpk, wpk_ext),
                         (obias, obias_ext), (maxv, maxv_ext),
                         (minv, minv_ext)]:
                nc.sync.dma_start(out=t[:], in_=e[:])

            # ---- main loop: 1024-col chunks, 2-tile software pipeline ----
            # PE program order interleaves stage-1 of tile i+1/i+2 between
            # stage-1(i) and stage-2(i) so the PE streams while the clamp
            # (DVE/ACT) catches up; without the lag the PE stalls ~500ns
            # per tile waiting for h.
            LAG = 2
            for cp in range(NS // (2 * CH)):
                oc_map = {}

                def emit_s2(g, t, h):
                    gi = g * TPG + t
                    oc = oc_map[g]
                    for half in range(2):
                        nc.tensor.matmul(
                            oc[:, half * CH:(half + 1) * CH],
                            wpk[:, gi * 128:(gi + 1) * 128],
                            h[:, half * CH:(half + 1) * CH],
                            start=(t == 0), stop=(t == TPG - 1),
                        )
                    if t == TPG - 1:
                        osb = opool.tile([128, 2 * CH], bf16, tag="osb",
                                         name="osb")
                        if g % 4 == 3:
                            nc.vector.tensor_scalar(osb[:], oc[:],
                                                    obias[:, g:g + 1], None,
                                                    aadd)
                        else:
                            nc.scalar.activation(osb[:], oc[:], Ident,
                                                 bias=obias[:, g:g + 1])
                        nc.sync.dma_start(
                            out=out_ext[g * 128:(g + 1) * 128,
                                        2 * cp * CH:2 * (cp + 1) * CH],
                            in_=osb[:])
                        del oc_map[g]

                pend = []
                for g in range(NG):
                    xt = xT[g // 2]
                    oc_map[g] = ocpool.tile([128, 2 * CH], f32, tag="oc",
                                            name="oc")
                    for t in range(TPG):
                        gi = g * TPG + t
                        ra = tile_route_a(gi)
                        ps = bcpool.tile([128, 2 * CH], f32, tag="ps")
                        for half in range(2):
                            c = 2 * cp + half
                            nc.tensor.matmul(
                                ps[:, half * CH:(half + 1) * CH],
                                selpk[:, gi * 128:(gi + 1) * 128],
                                xt[:, c * CH:(c + 1) * CH],
                                start=True, stop=True,
                            )
                        h = hpool.tile([128, 2 * CH], bf16, tag="h")
                        if ra:
                            nc.vector.tensor_scalar(
                                h[:], ps[:], maxv[:, gi:gi + 1], minv[:, gi:gi + 1],
                                amax, amin,
                            )
                        else:
                            r = spool.tile([128, 2 * CH], bf16, tag="r")
                            nc.scalar.activation(r[:], ps[:], Relu)
                            nc.vector.tensor_scalar(
                                h[:], r[:], minv[:, gi:gi + 1], None, amin,
                            )
                        pend.append((g, t, h))
                        if len(pend) > LAG:
                            emit_s2(*pend.pop(0))
                for args in pend:
                    emit_s2(*args)

    nc.compile()
    return nc


def host_constants(edges, width, W, b):
    """Build packed constant tensors. edges/width [F,B], W [F,B,D], b [F,D]."""
    f32 = np.float32
    edges = np.asarray(edges, f32)
    width = np.asarray(width, f32)
    W = np.asarray(W, f32).copy()
    b = np.asarray(b, f32)
    wv_all = bfr(1.0 / width)        # bf16-valued winv, f32
    e1 = edges[:, 0] + width[:, 0]   # second boundary

    selpk = np.zeros((128, NT * 128), f32)
    wpack = np.zeros((128, NT * 128), f32)
    obias = np.zeros((128, NG), f32)
    maxv = np.zeros((128, NT), f32)
    minv = np.zeros((128, NT), f32)

    for g in range(NG):
        ti, band = g // 2, 64 * (g % 2)
        for t in range(TPG):
            gi = g * TPG + t
            ra = tile_route_a(gi)
            selcol = gi * 128
            for m in range(128):
                r = 128 * t + m          # row within the group (0..767)
                fl, j = r // B, r % B    # local feature, bin
                f = 16 * g + fl
                wv = wv_all[f, j]
                wcoef = W[f, j, :]
                if j == 0 and not ra:
                    # relu-form: value = relu((e1-x)*winv), weight -W0,
                    # obias += W0
                    xw = -wv
                    cval = f32(e1[f] * wv)
                    minv[m, gi] = BIG
                    wcoef = -W[f, j, :]
                    obias[8 * fl:8 * fl + 8, g] += W[f, j, :]
                else:
                    xw = wv
                    cval = f32(-edges[f, j] * wv)
                    if j == 0:           # route A bin0: min(s,1) only
                        maxv[m, gi] = -BIG
                        minv[m, gi] = 1.0
                    elif j == B - 1:     # last bin: max(s,0) only
                        maxv[m, gi] = 0.0
                        minv[m, gi] = BIG
                    else:
                        maxv[m, gi] = 0.0
                        minv[m, gi] = 1.0
                chi = bfr(cval)
                clo = f32(cval - chi)
                selpk[band + fl, selcol + m] = xw
                selpk[band + 16 + fl, selcol + m] = xw
                selpk[band + 32, selcol + m] = chi
                selpk[band + 33, selcol + m] = clo
                wpack[m, gi * 128 + 8 * fl:gi * 128 + 8 * fl + 8] = wcoef
        for fl in range(16):
            obias[8 * fl:8 * fl + 8, g] += b[16 * g + fl, :]

    return {
        "selpk": selpk.astype(BF16),
        "wpack": wpack.astype(BF16),
        "obias": obias,
        "maxv": maxv,
        "minv": minv,
    }


def make_xT(x_core):
    """x_core [NS, F] f32 -> packed [8*128, NS] bf16 (hi/lo split bands)."""
    xT = np.zeros((8 * 128, NS), BF16)
    xt_full = np.ascontiguousarray(x_core.T).astype(np.float32)   # [F, NS]
    xhi = xt_full.astype(BF16)
    xlo = (xt_full - xhi.astype(np.float32)).astype(BF16)
    one = BF16(1.0)
    for g in range(NG):
        base = 128 * (g // 2) + 64 * (g % 2)
        xT[base:base + 16, :] = xhi[16 * g:16 * g + 16, :]
        xT[base + 16:base + 32, :] = xlo[16 * g:16 * g + 16, :]
        xT[base + 32, :] = one
        xT[base + 33, :] = one
    return xT


def make_in_maps(x, edges, width, W, b):
    consts = host_constants(edges, width, W, b)
    x = np.ascontiguousarray(np.asarray(x, dtype=np.float32))
    in_maps = []
    for core in range(NCORES):
        m = dict(consts)
        m["xT"] = make_xT(x[core * NS:(core + 1) * NS, :])
        in_maps.append(m)
    return in_maps


def kernel(x, edges, width, W, b):
    if "nc" not in _cache:
        _cache["nc"] = build_nc()
    nc = _cache["nc"]
    in_maps = make_in_maps(x, edges, width, W, b)
    res = run_bass_kernel_spmd(nc, in_maps, core_ids=list(range(NCORES)))
    outs = []
    for r in res.results:
        o = np.asarray(r["out"])                      # [F*D, NS] bf16
        outs.append(o.astype(np.float32).T)           # [NS, F*D]
    return np.ascontiguousarray(np.concatenate(outs, axis=0))


# revision 6
# speedup vs baseline: 1.6896x; 1.0626x over previous
"""Trainium2 Bass kernel for CompactPiecewiseLinearEmbeddings.

out[n, f*8+d] = sum_b h[n,f,b] * W[f,b,d] + b[f,d]
h = piecewise-linear encoding of x[n,f] over per-feature bins
    (first bin clamp_max(1), middle clamp(0,1), last bin clamp_min(0)).

Strategy (per core; data-parallel over N across 8 cores):
 - All-bf16 PE path (f32r moving data caps the PE clock at 1.2 GHz;
   pure-bf16 streams sustain 2.4 GHz).  x is split hi/lo into two bf16
   rows per feature (bf16*bf16 products are exact in the fp32 PSUM
   accumulate), and the per-bin bias -e*winv is split across two bf16
   ones-rows, so stage-1 matches f32r accuracy.
 - Host packs x into 8 xT tiles [128, NS]: two 34-row bands per tile
   (rows 0/64 +: 16 x_hi, 16 x_lo, ones, ones) serving groups (2i,2i+1).
 - Stage-1 bf16 matmul per (group, tile): s[f,j] = winv*x - e*winv.
   Contraction is always the full 128 partitions with zeros in the
   unused weight rows: mixing partial-band (tile_position) matmuls with
   full-128 ones drops the PE cadence from 216ns to ~322ns per matmul
   (measured), while uniform [128,128,512] shapes sustain the 2.4 GHz
   boost.
 - Clamp s -> h (bf16) via two static routes:
     A (~37%): DVE dual tensor_scalar (max, min) from PSUM.
     B: ACT Relu from PSUM (bin0 rows use the negated relu((e1-x)*winv)
        form with +W0 folded into the output bias), then DVE min-pass.
 - Stage-2 bf16 matmul contracts h against block-diagonal W into
   [128 fd, 1024] PSUM (6 accumulating matmuls per group).
 - Output bias+evac split ACT/DVE (3:1) -> bf16 SBUF -> DMA out in
   [fd, n] layout.  Host transposes/casts to the final [n, fd] f32.
"""
import numpy as np
import ml_dtypes

from concourse import bacc, mybir
from concourse.tile import TileContext
from concourse.bass_utils import run_bass_kernel_spmd

N, F, B, D = 16384, 256, 48, 8
NCORES = 8
NS = N // NCORES          # 2048 rows per core
CH = 512                  # matmul free dim (PSUM bank = 512 f32)
NG = 16                   # feature groups (16 features each)
TPG = 6                   # h-tiles per group (16*48/128)
NT = NG * TPG             # 96 h-tiles
NB = 34                   # band rows: 16 x_hi + 16 x_lo + 2 ones
BIG = 1e30

BF16 = ml_dtypes.bfloat16


def tile_route_a(gi):
    """Static clamp route per h-tile: A = DVE dual clamp, B = ACT relu
    + DVE min.  40% A, evenly spread so DVE-heavy tiles never cluster
    (clusters stall the PE on the ps-ring)."""
    return (gi % 10) in (0, 3, 5, 8)


def bfr(a):
    """Round f32 array to bf16 grid, keep f32."""
    return np.asarray(a, np.float32).astype(BF16).astype(np.float32)


_cache = {}


def build_nc():
    nc = bacc.Bacc("TRN2")
    f32, bf16 = mybir.dt.float32, mybir.dt.bfloat16

    xT_ext = nc.declare_dram_parameter("xT", [8 * 128, NS], bf16, isOutput=False)
    selpk_ext = nc.declare_dram_parameter("selpk", [128, NT * 128], bf16,
                                          isOutput=False)
    wpk_ext = nc.declare_dram_parameter("wpack", [128, NT * 128], bf16,
                                        isOutput=False)
    obias_ext = nc.declare_dram_parameter("obias", [128, NG], f32, isOutput=False)
    maxv_ext = nc.declare_dram_parameter("maxv", [128, NT], f32, isOutput=False)
    minv_ext = nc.declare_dram_parameter("minv", [128, NT], f32, isOutput=False)
    out_ext = nc.declare_dram_parameter("out", [F * D, NS], bf16, isOutput=True)

    Ident = mybir.ActivationFunctionType.Identity
    Relu = mybir.ActivationFunctionType.Relu
    amax, amin = mybir.AluOpType.max, mybir.AluOpType.min
    aadd = mybir.AluOpType.add

    with TileContext(nc) as tc:
        with (
            tc.tile_pool(name="const", bufs=1) as cpool,
            tc.tile_pool(name="hbuf", bufs=8) as hpool,
            tc.tile_pool(name="sbuf2", bufs=4) as spool,
            tc.tile_pool(name="osb", bufs=4) as opool,
            tc.tile_pool(name="bc", bufs=2, space="PSUM") as bcpool,
            tc.tile_pool(name="oc", bufs=2, space="PSUM") as ocpool,
        ):
            # ---- constants ----
            xT = [cpool.tile([128, NS], bf16, tag=f"xT{i}", name=f"xT{i}")
                  for i in range(8)]
            for i in range(8):
                nc.sync.dma_start(out=xT[i][:], in_=xT_ext[i * 128:(i + 1) * 128, :])
            selpk = cpool.tile([128, NT * 128], bf16)
            wpk = cpool.tile([128, NT * 128], bf16)
            obias = cpool.tile([128, NG], f32)
            maxv = cpool.tile([128, NT], f32)
            minv = cpool.tile([128, NT], f32)
            HALF = (NT // 2) * 128
            nc.sync.dma_start(out=selpk[:, 0:HALF], in_=selpk_ext[:, 0:HALF])
            for t, e in [(obias, obias_ext), (maxv, maxv_ext),
                         (minv, minv_ext)]:
                nc.sync.dma_start(out=t[:], in_=e[:])
            nc.sync.dma_start(out=wpk[:, 0:HALF], in_=wpk_ext[:, 0:HALF])
            nc.sync.dma_start(out=selpk[:, HALF:], in_=selpk_ext[:, HALF:])
            nc.sync.dma_start(out=wpk[:, HALF:], in_=wpk_ext[:, HALF:])

            # ---- main loop: 1024-col chunks, 2-tile software pipeline ----
            # PE program order interleaves stage-1 of tile i+1/i+2 between
            # stage-1(i) and stage-2(i) so the PE streams while the clamp
            # (DVE/ACT) catches up; without the lag the PE stalls ~500ns
            # per tile waiting for h.
            LAG = 3
            for cp in range(NS // (2 * CH)):
                oc_map = {}

                def emit_s2(g, t, h):
                    gi = g * TPG + t
                    oc = oc_map[g]
                    for half in range(2):
                        nc.tensor.matmul(
                            oc[:, half * CH:(half + 1) * CH],
                            wpk[:, gi * 128:(gi + 1) * 128],
                            h[:, half * CH:(half + 1) * CH],
                            start=(t == 0), stop=(t == TPG - 1),
                        )
                    if t == TPG - 1:
                        osb = opool.tile([128, 2 * CH], bf16, tag="osb",
                                         name="osb")
                        nc.scalar.activation(osb[:], oc[:], Ident,
                                             bias=obias[:, g:g + 1])
                        nc.sync.dma_start(
                            out=out_ext[g * 128:(g + 1) * 128,
                                        2 * cp * CH:2 * (cp + 1) * CH],
                            in_=osb[:])
                        del oc_map[g]

                pend = []
                for g in range(NG):
                    xt = xT[g // 2]
                    oc_map[g] = ocpool.tile([128, 2 * CH], f32, tag="oc",
                                            name="oc")
                    for t in range(TPG):
                        gi = g * TPG + t
                        ra = tile_route_a(gi)
                        ps = bcpool.tile([128, 2 * CH], f32, tag="ps")
                        for half in range(2):
                            c = 2 * cp + half
                            nc.tensor.matmul(
                                ps[:, half * CH:(half + 1) * CH],
                                selpk[:, gi * 128:(gi + 1) * 128],
                                xt[:, c * CH:(c + 1) * CH],
                                start=True, stop=True,
                            )
                        h = hpool.tile([128, 2 * CH], bf16, tag="h")
                        if ra:
                            nc.vector.tensor_scalar(
                                h[:], ps[:], maxv[:, gi:gi + 1], minv[:, gi:gi + 1],
                                amax, amin,
                            )
                        else:
                            r = spool.tile([128, 2 * CH], bf16, tag="r")
                            nc.scalar.activation(r[:], ps[:], Relu)
                            nc.vector.tensor_scalar(
                                h[:], r[:], minv[:, gi:gi + 1], None, amin,
                            )
                        pend.append((g, t, h))
                        if len(pend) > LAG:
                            emit_s2(*pend.pop(0))
                for args in pend:
                    emit_s2(*args)

    nc.compile()
    return nc


def host_constants(edges, width, W, b):
    """Build packed constant tensors. edges/width [F,B], W [F,B,D], b [F,D]."""
    f32 = np.float32
    edges = np.asarray(edges, f32)
    width = np.asarray(width, f32)
    W = np.asarray(W, f32).copy()
    b = np.asarray(b, f32)
    wv_all = bfr(1.0 / width)        # bf16-valued winv, f32
    e1 = edges[:, 0] + width[:, 0]   # second boundary

    selpk = np.zeros((128, NT * 128), f32)
    wpack = np.zeros((128, NT * 128), f32)
    obias = np.zeros((128, NG), f32)
    maxv = np.zeros((128, NT), f32)
    minv = np.zeros((128, NT), f32)

    for g in range(NG):
        ti, band = g // 2, 64 * (g % 2)
        for t in range(TPG):
            gi = g * TPG + t
            ra = tile_route_a(gi)
            selcol = gi * 128
            for m in range(128):
                r = 128 * t + m          # row within the group (0..767)
                fl, j = r // B, r % B    # local feature, bin
                f = 16 * g + fl
                wv = wv_all[f, j]
                wcoef = W[f, j, :]
                if j == 0 and not ra:
                    # relu-form: value = relu((e1-x)*winv), weight -W0,
                    # obias += W0
                    xw = -wv
                    cval = f32(e1[f] * wv)
                    minv[m, gi] = BIG
                    wcoef = -W[f, j, :]
                    obias[8 * fl:8 * fl + 8, g] += W[f, j, :]
                else:
                    xw = wv
                    cval = f32(-edges[f, j] * wv)
                    if j == 0:           # route A bin0: min(s,1) only
                        maxv[m, gi] = -BIG
                        minv[m, gi] = 1.0
                    elif j == B - 1:     # last bin: max(s,0) only
                        maxv[m, gi] = 0.0
                        minv[m, gi] = BIG
                    else:
                        maxv[m, gi] = 0.0
                        minv[m, gi] = 1.0
                chi = bfr(cval)
                clo = f32(cval - chi)
                selpk[band + fl, selcol + m] = xw
                selpk[band + 16 + fl, selcol + m] = xw
                selpk[band + 32, selcol + m] = chi
                selpk[band + 33, selcol + m] = clo
                wpack[m, gi * 128 + 8 * fl:gi * 128 + 8 * fl + 8] = wcoef
        for fl in range(16):
            obias[8 * fl:8 * fl + 8, g] += b[16 * g + fl, :]

    return {
        "selpk": selpk.astype(BF16),
        "wpack": wpack.astype(BF16),
        "obias": obias,
        "maxv": maxv,
        "minv": minv,
    }


def make_xT(x_core):
    """x_core [NS, F] f32 -> packed [8*128, NS] bf16 (hi/lo split bands)."""
    xT = np.zeros((8 * 128, NS), BF16)
    xt_full = np.ascontiguousarray(x_core.T).astype(np.float32)   # [F, NS]
    xhi = xt_full.astype(BF16)
    xlo = (xt_full - xhi.astype(np.float32)).astype(BF16)
    one = BF16(1.0)
    for g in range(NG):
        base = 128 * (g // 2) + 64 * (g % 2)
        xT[base:base + 16, :] = xhi[16 * g:16 * g + 16, :]
        xT[base + 16:base + 32, :] = xlo[16 * g:16 * g + 16, :]
        xT[base + 32, :] = one
        xT[base + 33, :] = one
    return xT


def make_in_maps(x, edges, width, W, b):
    consts = host_constants(edges, width, W, b)
    x = np.ascontiguousarray(np.asarray(x, dtype=np.float32))
    in_maps = []
    for core in range(NCORES):
        m = dict(consts)
        m["xT"] = make_xT(x[core * NS:(core + 1) * NS, :])
        in_maps.append(m)
    return in_maps


def kernel(x, edges, width, W, b):
    if "nc" not in _cache:
        _cache["nc"] = build_nc()
    nc = _cache["nc"]
    in_maps = make_in_maps(x, edges, width, W, b)
    res = run_bass_kernel_spmd(nc, in_maps, core_ids=list(range(NCORES)))
    outs = []
    for r in res.results:
        o = np.asarray(r["out"])                      # [F*D, NS] bf16
        outs.append(o.astype(np.float32).T)           # [NS, F*D]
    return np.ascontiguousarray(np.concatenate(outs, axis=0))


# revision 7
# speedup vs baseline: 2.0305x; 1.2018x over previous
"""Trainium2 Bass kernel for CompactPiecewiseLinearEmbeddings.

out[n, f*8+d] = sum_b h[n,f,b] * W[f,b,d] + b[f,d]
h = piecewise-linear encoding of x[n,f] over per-feature bins
    (first bin clamp_max(1), middle clamp(0,1), last bin clamp_min(0)).

Strategy (per core; data-parallel over N across 8 cores):
 - All-bf16 PE path (f32r moving data caps the PE clock at 1.2 GHz;
   pure-bf16 streams sustain 2.4 GHz).  x is split hi/lo into two bf16
   rows per feature (bf16*bf16 products are exact in the fp32 PSUM
   accumulate), and the per-bin bias -e*winv is split across two bf16
   ones-rows, so stage-1 matches f32r accuracy.
 - Host packs x into 8 xT tiles [128, NS]: two 34-row bands per tile
   (rows 0/64 +: 16 x_hi, 16 x_lo, ones, ones) serving groups (2i,2i+1).
 - Stage-1 bf16 matmul per (group, tile): s[f,j] = winv*x - e*winv.
   Contraction is always the full 128 partitions with zeros in the
   unused weight rows: mixing partial-band (tile_position) matmuls with
   full-128 ones drops the PE cadence from 216ns to ~322ns per matmul
   (measured), while uniform [128,128,512] shapes sustain the 2.4 GHz
   boost.
 - Clamp s -> h (bf16) via two static routes:
     A (~37%): DVE dual tensor_scalar (max, min) from PSUM.
     B: ACT Relu from PSUM (bin0 rows use the negated relu((e1-x)*winv)
        form with +W0 folded into the output bias), then DVE min-pass.
 - Stage-2 bf16 matmul contracts h against block-diagonal W into
   [128 fd, 1024] PSUM (6 accumulating matmuls per group).
 - Output bias+evac split ACT/DVE (3:1) -> bf16 SBUF -> DMA out in
   [fd, n] layout.  Host transposes/casts to the final [n, fd] f32.
"""
import numpy as np
import ml_dtypes

from concourse import bacc, mybir
from concourse.tile import TileContext
from concourse.bass_utils import run_bass_kernel_spmd

N, F, B, D = 16384, 256, 48, 8
NCORES = 8
NS = N // NCORES          # 2048 rows per core
CH = 512                  # matmul free dim (PSUM bank = 512 f32)
NG = 16                   # feature groups (16 features each)
TPG = 6                   # h-tiles per group (16*48/128)
NT = NG * TPG             # 96 h-tiles
NB = 34                   # band rows: 16 x_hi + 16 x_lo + 2 ones
BIG = 1e30

BF16 = ml_dtypes.bfloat16


def tile_route_a(gi):
    """Static clamp route per h-tile: A = DVE dual clamp, B = ACT relu
    + DVE min.  40% A, evenly spread so DVE-heavy tiles never cluster
    (clusters stall the PE on the ps-ring)."""
    return (gi % 10) in (0, 3, 5, 8)


def bfr(a):
    """Round f32 array to bf16 grid, keep f32."""
    return np.asarray(a, np.float32).astype(BF16).astype(np.float32)


_cache = {}


def build_nc():
    nc = bacc.Bacc("TRN2")
    f32, bf16 = mybir.dt.float32, mybir.dt.bfloat16

    xT_ext = nc.declare_dram_parameter("xT", [8 * 128, NS], bf16, isOutput=False)
    selpk_ext = nc.declare_dram_parameter("selpk", [128, NT * 128], bf16,
                                          isOutput=False)
    wpk_ext = nc.declare_dram_parameter("wpack", [128, NT * 128], bf16,
                                        isOutput=False)
    obias_ext = nc.declare_dram_parameter("obias", [128, NG], f32, isOutput=False)
    maxv_ext = nc.declare_dram_parameter("maxv", [128, NT], f32, isOutput=False)
    minv_ext = nc.declare_dram_parameter("minv", [128, NT], f32, isOutput=False)
    out_ext = nc.declare_dram_parameter("out", [F * D, NS], bf16, isOutput=True)

    Ident = mybir.ActivationFunctionType.Identity
    Relu = mybir.ActivationFunctionType.Relu
    amax, amin = mybir.AluOpType.max, mybir.AluOpType.min
    aadd = mybir.AluOpType.add

    with TileContext(nc) as tc:
        with (
            tc.tile_pool(name="const", bufs=1) as cpool,
            tc.tile_pool(name="hbuf", bufs=8) as hpool,
            tc.tile_pool(name="sbuf2", bufs=4) as spool,
            tc.tile_pool(name="osb", bufs=4) as opool,
            tc.tile_pool(name="bc", bufs=3, space="PSUM") as bcpool,
            tc.tile_pool(name="oc", bufs=1, space="PSUM") as ocpool,
        ):
            # ---- constants ----
            xT = [cpool.tile([128, NS], bf16, tag=f"xT{i}", name=f"xT{i}")
                  for i in range(8)]
            for i in range(8):
                nc.sync.dma_start(out=xT[i][:], in_=xT_ext[i * 128:(i + 1) * 128, :])
            selpk = cpool.tile([128, NT * 128], bf16)
            wpk = cpool.tile([128, NT * 128], bf16)
            obias = cpool.tile([128, NG], f32)
            maxv = cpool.tile([128, NT], f32)
            minv = cpool.tile([128, NT], f32)
            HALF = (NT // 2) * 128
            nc.sync.dma_start(out=selpk[:, 0:HALF], in_=selpk_ext[:, 0:HALF])
            for t, e in [(obias, obias_ext), (maxv, maxv_ext),
                         (minv, minv_ext)]:
                nc.sync.dma_start(out=t[:], in_=e[:])
            nc.sync.dma_start(out=wpk[:, 0:HALF], in_=wpk_ext[:, 0:HALF])
            nc.sync.dma_start(out=selpk[:, HALF:], in_=selpk_ext[:, HALF:])
            nc.sync.dma_start(out=wpk[:, HALF:], in_=wpk_ext[:, HALF:])

            # ---- main loop: 1024-col chunks, 2-tile software pipeline ----
            # PE program order interleaves stage-1 of tile i+1/i+2 between
            # stage-1(i) and stage-2(i) so the PE streams while the clamp
            # (DVE/ACT) catches up; without the lag the PE stalls ~500ns
            # per tile waiting for h.
            LAG = 3
            for cp in range(NS // (2 * CH)):
                oc_map = {}

                def emit_s2(g, t, h):
                    gi = g * TPG + t
                    oc2 = oc_map[g]
                    for half in range(2):
                        nc.tensor.matmul(
                            oc2[half][:],
                            wpk[:, gi * 128:(gi + 1) * 128],
                            h[:, half * CH:(half + 1) * CH],
                            start=(t == 0), stop=(t == TPG - 1),
                        )
                    if t == TPG - 1:
                        osb = opool.tile([128, 2 * CH], bf16, tag="osb",
                                         name="osb")
                        for half in range(2):
                            nc.scalar.activation(
                                osb[:, half * CH:(half + 1) * CH],
                                oc2[half][:], Ident,
                                bias=obias[:, g:g + 1])
                        nc.sync.dma_start(
                            out=out_ext[g * 128:(g + 1) * 128,
                                        2 * cp * CH:2 * (cp + 1) * CH],
                            in_=osb[:])
                        del oc_map[g]

                pend = []
                for g in range(NG):
                    xt = xT[g // 2]
                    oc_map[g] = (ocpool.tile([128, CH], f32, tag="oca",
                                             name="oca"),
                                 ocpool.tile([128, CH], f32, tag="ocb",
                                             name="ocb"))
                    for t in range(TPG):
                        gi = g * TPG + t
                        ra = tile_route_a(gi)
                        ps = bcpool.tile([128, 2 * CH], f32, tag="ps")
                        for half in range(2):
                            c = 2 * cp + half
                            nc.tensor.matmul(
                                ps[:, half * CH:(half + 1) * CH],
                                selpk[:, gi * 128:(gi + 1) * 128],
                                xt[:, c * CH:(c + 1) * CH],
                                start=True, stop=True,
                            )
                        h = hpool.tile([128, 2 * CH], bf16, tag="h")
                        if ra:
                            nc.vector.tensor_scalar(
                                h[:], ps[:], maxv[:, gi:gi + 1], minv[:, gi:gi + 1],
                                amax, amin,
                            )
                        else:
                            r = spool.tile([128, 2 * CH], bf16, tag="r")
                            nc.scalar.activation(r[:], ps[:], Relu)
                            nc.vector.tensor_scalar(
                                h[:], r[:], minv[:, gi:gi + 1], None, amin,
                            )
                        pend.append((g, t, h))
                        if len(pend) > LAG:
                            emit_s2(*pend.pop(0))
                for args in pend:
                    emit_s2(*args)

    nc.compile()
    return nc


def host_constants(edges, width, W, b):
    """Build packed constant tensors. edges/width [F,B], W [F,B,D], b [F,D]."""
    f32 = np.float32
    edges = np.asarray(edges, f32)
    width = np.asarray(width, f32)
    W = np.asarray(W, f32).copy()
    b = np.asarray(b, f32)
    wv_all = bfr(1.0 / width)        # bf16-valued winv, f32
    e1 = edges[:, 0] + width[:, 0]   # second boundary

    selpk = np.zeros((128, NT * 128), f32)
    wpack = np.zeros((128, NT * 128), f32)
    obias = np.zeros((128, NG), f32)
    maxv = np.zeros((128, NT), f32)
    minv = np.zeros((128, NT), f32)

    for g in range(NG):
        ti, band = g // 2, 64 * (g % 2)
        for t in range(TPG):
            gi = g * TPG + t
            ra = tile_route_a(gi)
            selcol = gi * 128
            for m in range(128):
                r = 128 * t + m          # row within the group (0..767)
                fl, j = r // B, r % B    # local feature, bin
                f = 16 * g + fl
                wv = wv_all[f, j]
                wcoef = W[f, j, :]
                if j == 0 and not ra:
                    # relu-form: value = relu((e1-x)*winv), weight -W0,
                    # obias += W0
                    xw = -wv
                    cval = f32(e1[f] * wv)
                    minv[m, gi] = BIG
                    wcoef = -W[f, j, :]
                    obias[8 * fl:8 * fl + 8, g] += W[f, j, :]
                else:
                    xw = wv
                    cval = f32(-edges[f, j] * wv)
                    if j == 0:           # route A bin0: min(s,1) only
                        maxv[m, gi] = -BIG
                        minv[m, gi] = 1.0
                    elif j == B - 1:     # last bin: max(s,0) only
                        maxv[m, gi] = 0.0
                        minv[m, gi] = BIG
                    else:
                        maxv[m, gi] = 0.0
                        minv[m, gi] = 1.0
                chi = bfr(cval)
                clo = f32(cval - chi)
                selpk[band + fl, selcol + m] = xw
                selpk[band + 16 + fl, selcol + m] = xw
                selpk[band + 32, selcol + m] = chi
                selpk[band + 33, selcol + m] = clo
                wpack[m, gi * 128 + 8 * fl:gi * 128 + 8 * fl + 8] = wcoef
        for fl in range(16):
            obias[8 * fl:8 * fl + 8, g] += b[16 * g + fl, :]

    return {
        "selpk": selpk.astype(BF16),
        "wpack": wpack.astype(BF16),
        "obias": obias,
        "maxv": maxv,
        "minv": minv,
    }


def make_xT(x_core):
    """x_core [NS, F] f32 -> packed [8*128, NS] bf16 (hi/lo split bands)."""
    xT = np.zeros((8 * 128, NS), BF16)
    xt_full = np.ascontiguousarray(x_core.T).astype(np.float32)   # [F, NS]
    xhi = xt_full.astype(BF16)
    xlo = (xt_full - xhi.astype(np.float32)).astype(BF16)
    one = BF16(1.0)
    for g in range(NG):
        base = 128 * (g // 2) + 64 * (g % 2)
        xT[base:base + 16, :] = xhi[16 * g:16 * g + 16, :]
        xT[base + 16:base + 32, :] = xlo[16 * g:16 * g + 16, :]
        xT[base + 32, :] = one
        xT[base + 33, :] = one
    return xT


def make_in_maps(x, edges, width, W, b):
    consts = host_constants(edges, width, W, b)
    x = np.ascontiguousarray(np.asarray(x, dtype=np.float32))
    in_maps = []
    for core in range(NCORES):
        m = dict(consts)
        m["xT"] = make_xT(x[core * NS:(core + 1) * NS, :])
        in_maps.append(m)
    return in_maps


def kernel(x, edges, width, W, b):
    if "nc" not in _cache:
        _cache["nc"] = build_nc()
    nc = _cache["nc"]
    in_maps = make_in_maps(x, edges, width, W, b)
    res = run_bass_kernel_spmd(nc, in_maps, core_ids=list(range(NCORES)))
    outs = []
    for r in res.results:
        o = np.asarray(r["out"])                      # [F*D, NS] bf16
        outs.append(o.astype(np.float32).T)           # [NS, F*D]
    return np.ascontiguousarray(np.concatenate(outs, axis=0))


# revision 8
# speedup vs baseline: 2.0509x; 1.0100x over previous
"""Trainium2 Bass kernel for CompactPiecewiseLinearEmbeddings.

out[n, f*8+d] = sum_b h[n,f,b] * W[f,b,d] + b[f,d]
h = piecewise-linear encoding of x[n,f] over per-feature bins
    (first bin clamp_max(1), middle clamp(0,1), last bin clamp_min(0)).

Strategy (per core; data-parallel over N across 8 cores):
 - All-bf16 PE path (f32r moving data caps the PE clock at 1.2 GHz;
   pure-bf16 streams sustain 2.4 GHz).  x is split hi/lo into two bf16
   rows per feature (bf16*bf16 products are exact in the fp32 PSUM
   accumulate), and the per-bin bias -e*winv is split across two bf16
   ones-rows, so stage-1 matches f32r accuracy.
 - Host packs x into 8 xT tiles [128, NS]: two 34-row bands per tile
   (rows 0/64 +: 16 x_hi, 16 x_lo, ones, ones) serving groups (2i,2i+1).
 - Stage-1 bf16 matmul per (group, tile): s[f,j] = winv*x - e*winv.
   Contraction is always the full 128 partitions with zeros in the
   unused weight rows: mixing partial-band (tile_position) matmuls with
   full-128 ones drops the PE cadence from 216ns to ~322ns per matmul
   (measured), while uniform [128,128,512] shapes sustain the 2.4 GHz
   boost.
 - Clamp s -> h (bf16) via two static routes:
     A (~37%): DVE dual tensor_scalar (max, min) from PSUM.
     B: ACT Relu from PSUM (bin0 rows use the negated relu((e1-x)*winv)
        form with +W0 folded into the output bias), then DVE min-pass.
 - Stage-2 bf16 matmul contracts h against block-diagonal W into
   [128 fd, 1024] PSUM (6 accumulating matmuls per group).
 - Output bias+evac split ACT/DVE (3:1) -> bf16 SBUF -> DMA out in
   [fd, n] layout.  Host transposes/casts to the final [n, fd] f32.
"""
import numpy as np
import ml_dtypes

from concourse import bacc, mybir
from concourse.tile import TileContext
from concourse.bass_utils import run_bass_kernel_spmd

N, F, B, D = 16384, 256, 48, 8
NCORES = 8
NS = N // NCORES          # 2048 rows per core
CH = 512                  # matmul free dim (PSUM bank = 512 f32)
NG = 16                   # feature groups (16 features each)
TPG = 6                   # h-tiles per group (16*48/128)
NT = NG * TPG             # 96 h-tiles
NB = 34                   # band rows: 16 x_hi + 16 x_lo + 2 ones
BIG = 1e30

BF16 = ml_dtypes.bfloat16


def tile_route_a(gi):
    """Static clamp route per h-tile: A = DVE dual clamp, B = ACT relu
    + DVE min.  40% A, evenly spread so DVE-heavy tiles never cluster
    (clusters stall the PE on the ps-ring)."""
    return (gi % 10) in (0, 3, 5, 8)


def bfr(a):
    """Round f32 array to bf16 grid, keep f32."""
    return np.asarray(a, np.float32).astype(BF16).astype(np.float32)


_cache = {}


def build_nc():
    nc = bacc.Bacc("TRN2")
    f32, bf16 = mybir.dt.float32, mybir.dt.bfloat16

    xT_ext = nc.declare_dram_parameter("xT", [8 * 128, NS], bf16, isOutput=False)
    selpk_ext = nc.declare_dram_parameter("selpk", [128, NT * 128], bf16,
                                          isOutput=False)
    wpk_ext = nc.declare_dram_parameter("wpack", [128, NT * 128], bf16,
                                        isOutput=False)
    obias_ext = nc.declare_dram_parameter("obias", [128, NG], f32, isOutput=False)
    maxv_ext = nc.declare_dram_parameter("maxv", [128, NT], f32, isOutput=False)
    minv_ext = nc.declare_dram_parameter("minv", [128, NT], f32, isOutput=False)
    out_ext = nc.declare_dram_parameter("out", [F * D, NS], bf16, isOutput=True)

    Ident = mybir.ActivationFunctionType.Identity
    Relu = mybir.ActivationFunctionType.Relu
    amax, amin = mybir.AluOpType.max, mybir.AluOpType.min
    aadd = mybir.AluOpType.add

    with TileContext(nc) as tc:
        with (
            tc.tile_pool(name="const", bufs=1) as cpool,
            tc.tile_pool(name="hbuf", bufs=8) as hpool,
            tc.tile_pool(name="sbuf2", bufs=4) as spool,
            tc.tile_pool(name="osb", bufs=4) as opool,
            tc.tile_pool(name="bc", bufs=3, space="PSUM") as bcpool,
            tc.tile_pool(name="oc", bufs=1, space="PSUM") as ocpool,
        ):
            # ---- constants ----
            xT = [cpool.tile([128, NS], bf16, tag=f"xT{i}", name=f"xT{i}")
                  for i in range(8)]
            # chunked const tiles, DMA-ordered by first use (deps are
            # tile-granular: one big tile would stall the first matmul on
            # the whole transfer)
            NCHK = NT // 4                      # 24 h-tile blocks per chunk
            selpk4 = [cpool.tile([128, NCHK * 128], bf16, tag=f"selpk{i}",
                                 name=f"selpk{i}") for i in range(4)]
            wpk4 = [cpool.tile([128, NCHK * 128], bf16, tag=f"wpk{i}",
                               name=f"wpk{i}") for i in range(4)]
            obias = cpool.tile([128, NG], f32)
            maxv = cpool.tile([128, NT], f32)
            minv = cpool.tile([128, NT], f32)
            nc.sync.dma_start(out=maxv[:], in_=maxv_ext[:])
            nc.sync.dma_start(out=minv[:], in_=minv_ext[:])
            CW = NCHK * 128
            order = []
            for i in range(4):
                order.append((selpk4[i], selpk_ext, i))
                order.append((xT[2 * i], None, 2 * i))
                order.append((wpk4[i], wpk_ext, i))
                order.append((xT[2 * i + 1], None, 2 * i + 1))
            for tdst, ext, i in order:
                if ext is None:
                    nc.sync.dma_start(out=tdst[:],
                                      in_=xT_ext[i * 128:(i + 1) * 128, :])
                else:
                    nc.sync.dma_start(out=tdst[:],
                                      in_=ext[:, i * CW:(i + 1) * CW])
            nc.sync.dma_start(out=obias[:], in_=obias_ext[:])

            def selpk_blk(gi):
                return selpk4[gi // NCHK][:, (gi % NCHK) * 128:
                                          (gi % NCHK) * 128 + 128]

            def wpk_blk(gi):
                return wpk4[gi // NCHK][:, (gi % NCHK) * 128:
                                        (gi % NCHK) * 128 + 128]

            # ---- main loop: 1024-col chunks, 2-tile software pipeline ----
            # PE program order interleaves stage-1 of tile i+1/i+2 between
            # stage-1(i) and stage-2(i) so the PE streams while the clamp
            # (DVE/ACT) catches up; without the lag the PE stalls ~500ns
            # per tile waiting for h.
            LAG = 3
            for cp in range(NS // (2 * CH)):
                oc_map = {}

                def emit_s2(g, t, h):
                    gi = g * TPG + t
                    oc2 = oc_map[g]
                    for half in range(2):
                        nc.tensor.matmul(
                            oc2[half][:],
                            wpk_blk(gi),
                            h[:, half * CH:(half + 1) * CH],
                            start=(t == 0), stop=(t == TPG - 1),
                        )
                    if t == TPG - 1:
                        osb = opool.tile([128, 2 * CH], bf16, tag="osb",
                                         name="osb")
                        for half in range(2):
                            nc.scalar.activation(
                                osb[:, half * CH:(half + 1) * CH],
                                oc2[half][:], Ident,
                                bias=obias[:, g:g + 1])
                        nc.sync.dma_start(
                            out=out_ext[g * 128:(g + 1) * 128,
                                        2 * cp * CH:2 * (cp + 1) * CH],
                            in_=osb[:])
                        del oc_map[g]

                pend = []
                for g in range(NG):
                    xt = xT[g // 2]
                    oc_map[g] = (ocpool.tile([128, CH], f32, tag="oca",
                                             name="oca"),
                                 ocpool.tile([128, CH], f32, tag="ocb",
                                             name="ocb"))
                    for t in range(TPG):
                        gi = g * TPG + t
                        ra = tile_route_a(gi)
                        ps = bcpool.tile([128, 2 * CH], f32, tag="ps")
                        for half in range(2):
                            c = 2 * cp + half
                            nc.tensor.matmul(
                                ps[:, half * CH:(half + 1) * CH],
                                selpk_blk(gi),
                                xt[:, c * CH:(c + 1) * CH],
                                start=True, stop=True,
                            )
                        h = hpool.tile([128, 2 * CH], bf16, tag="h")
                        if ra:
                            nc.vector.tensor_scalar(
                                h[:], ps[:], maxv[:, gi:gi + 1], minv[:, gi:gi + 1],
                                amax, amin,
                            )
                        else:
                            r = spool.tile([128, 2 * CH], bf16, tag="r")
                            nc.scalar.activation(r[:], ps[:], Relu)
                            nc.vector.tensor_scalar(
                                h[:], r[:], minv[:, gi:gi + 1], None, amin,
                            )
                        pend.append((g, t, h))
                        if len(pend) > LAG:
                            emit_s2(*pend.pop(0))
                for args in pend:
                    emit_s2(*args)

    nc.compile()
    return nc


def host_constants(edges, width, W, b):
    """Build packed constant tensors. edges/width [F,B], W [F,B,D], b [F,D]."""
    f32 = np.float32
    edges = np.asarray(edges, f32)
    width = np.asarray(width, f32)
    W = np.asarray(W, f32).copy()
    b = np.asarray(b, f32)
    wv_all = bfr(1.0 / width)        # bf16-valued winv, f32
    e1 = edges[:, 0] + width[:, 0]   # second boundary

    selpk = np.zeros((128, NT * 128), f32)
    wpack = np.zeros((128, NT * 128), f32)
    obias = np.zeros((128, NG), f32)
    maxv = np.zeros((128, NT), f32)
    minv = np.zeros((128, NT), f32)

    for g in range(NG):
        ti, band = g // 2, 64 * (g % 2)
        for t in range(TPG):
            gi = g * TPG + t
            ra = tile_route_a(gi)
            selcol = gi * 128
            for m in range(128):
                r = 128 * t + m          # row within the group (0..767)
                fl, j = r // B, r % B    # local feature, bin
                f = 16 * g + fl
                wv = wv_all[f, j]
                wcoef = W[f, j, :]
                if j == 0 and not ra:
                    # relu-form: value = relu((e1-x)*winv), weight -W0,
                    # obias += W0
                    xw = -wv
                    cval = f32(e1[f] * wv)
                    minv[m, gi] = BIG
                    wcoef = -W[f, j, :]
                    obias[8 * fl:8 * fl + 8, g] += W[f, j, :]
                else:
                    xw = wv
                    cval = f32(-edges[f, j] * wv)
                    if j == 0:           # route A bin0: min(s,1) only
                        maxv[m, gi] = -BIG
                        minv[m, gi] = 1.0
                    elif j == B - 1:     # last bin: max(s,0) only
                        maxv[m, gi] = 0.0
                        minv[m, gi] = BIG
                    else:
                        maxv[m, gi] = 0.0
                        minv[m, gi] = 1.0
                chi = bfr(cval)
                clo = f32(cval - chi)
                selpk[band + fl, selcol + m] = xw
                selpk[band + 16 + fl, selcol + m] = xw
                selpk[band + 32, selcol + m] = chi
                selpk[band + 33, selcol + m] = clo
                wpack[m, gi * 128 + 8 * fl:gi * 128 + 8 * fl + 8] = wcoef
        for fl in range(16):
            obias[8 * fl:8 * fl + 8, g] += b[16 * g + fl, :]

    return {
        "selpk": selpk.astype(BF16),
        "wpack": wpack.astype(BF16),
        "obias": obias,
        "maxv": maxv,
        "minv": minv,
    }


def make_xT(x_core):
    """x_core [NS, F] f32 -> packed [8*128, NS] bf16 (hi/lo split bands)."""
    xT = np.zeros((8 * 128, NS), BF16)
    xt_full = np.ascontiguousarray(x_core.T).astype(np.float32)   # [F, NS]
    xhi = xt_full.astype(BF16)
    xlo = (xt_full - xhi.astype(np.float32)).astype(BF16)
    one = BF16(1.0)
    for g in range(NG):
        base = 128 * (g // 2) + 64 * (g % 2)
        xT[base:base + 16, :] = xhi[16 * g:16 * g + 16, :]
        xT[base + 16:base + 32, :] = xlo[16 * g:16 * g + 16, :]
        xT[base + 32, :] = one
        xT[base + 33, :] = one
    return xT


def make_in_maps(x, edges, width, W, b):
    consts = host_constants(edges, width, W, b)
    x = np.ascontiguousarray(np.asarray(x, dtype=np.float32))
    in_maps = []
    for core in range(NCORES):
        m = dict(consts)
        m["xT"] = make_xT(x[core * NS:(core + 1) * NS, :])
        in_maps.append(m)
    return in_maps


def kernel(x, edges, width, W, b):
    if "nc" not in _cache:
        _cache["nc"] = build_nc()
    nc = _cache["nc"]
    in_maps = make_in_maps(x, edges, width, W, b)
    res = run_bass_kernel_spmd(nc, in_maps, core_ids=list(range(NCORES)))
    outs = []
    for r in res.results:
        o = np.asarray(r["out"])                      # [F*D, NS] bf16
        outs.append(o.astype(np.float32).T)           # [NS, F*D]
    return np.ascontiguousarray(np.concatenate(outs, axis=0))
